# revision 17
# baseline (speedup 1.0000x reference)
"""v6: host-precomputed trig tables + vector-lean feature pipeline.

Structure (per core = one head, 4 sequences):
  - All trig tables (double-angle FM/LM fp16, chunk-0 exact fp32, omega/2pi
    fp16) are computed host-side from omega and DMA'd in; no on-device
    table generation.
  - Feature phase split per sequence into an Exp pass (E) and Trig pass (T)
    so the scalar engine loads each activation table set once per sequence.
  - elu1 fused as min(exp(x),1)+max(x,0) via TS(min)+TT(add); exp reads the
    raw fp32 window directly (inf clamps through min).
  - fp32->fp16 casts via tensor_scalar (tensor_copy falls to 1x mode).
  - GpSimd: sequence-wide TTs for ksl/ksf/qts/qtc + chunk-0 branch TTs.
  - Scan: 3 branch P matmuls -> masked p_sb -> intra/inter O matmuls; 3
    states share one PSUM bank with a single fused fp16 evacuation.
  - Scan of seq n-1 interleaved 2 chunks per feature sub-slot of seq n.
"""

import math

import numpy as np

import concourse.bass as bass
import concourse.tile as tile
from concourse import bacc, mybir
from concourse.bass_utils import run_bass_kernel_spmd
from concourse.masks import make_identity

F32 = mybir.dt.float32
F16 = mybir.dt.float16
AF = mybir.ActivationFunctionType
OP = mybir.AluOpType

N, L, H, D = 4, 2048, 8, 128
C = 128
NCH = L // C
DV1 = D + 1
VST = 130  # per-chunk slot width in v staging (4B-aligned stride)
SW = 130  # state slice stride in PSUM cols (8B aligned)
TWO_PI = 2.0 * math.pi
MAGIC = float(np.float32(1.5 * 2**23))
LN2 = float(np.log(2.0))
EPS = 1e-6
W = 512
CPW = W // C
NWIN = L // W

_CACHE = {}

_FM_TABLES = ["c2t_fm", "s2t_fm", "c2n_fm", "s2n_fm"]
_LM_TABLES = ["c2t_lm", "s2t_lm"]
_C0_TABLES = ["qs2_0", "qc2_0", "qsc_0", "kc2_0", "ks2_0", "ksc_0"]


def build_nc(n_seq=N, nch=NCH):
    l_eff = nch * C
    nc = bacc.Bacc(None, target_bir_lowering=False, debug=False)

    q_ext = nc.declare_dram_parameter("queries", [n_seq, nch, C, D], F32, isOutput=False)
    q2_ext = nc.declare_dram_parameter("q2", [n_seq, nch, C, D], F32, isOutput=False)
    k_ext = nc.declare_dram_parameter("keys", [n_seq, nch, C, D], F32, isOutput=False)
    v_ext = nc.declare_dram_parameter("values", [n_seq, nch, C, D], F32, isOutput=False)
    om_ext = nc.declare_dram_parameter("omega16", [D, D], F16, isOutput=False)
    mask_ext = nc.declare_dram_parameter("mask", [C, C], F16, isOutput=False)
    fm_ext = {t: nc.declare_dram_parameter(t, [D, l_eff], F16, isOutput=False) for t in _FM_TABLES}
    lm_ext = {t: nc.declare_dram_parameter(t, [C, nch * D], F16, isOutput=False) for t in _LM_TABLES}
    c0_ext = {t: nc.declare_dram_parameter(t, [D, C], F32, isOutput=False) for t in _C0_TABLES}
    out_ext = nc.declare_dram_parameter("out", [n_seq, NWIN, C, CPW * DV1], F16, isOutput=True)

    with tile.TileContext(nc) as tc:
        with (
            tc.tile_pool(name="persist", bufs=1) as pp,
            tc.tile_pool(name="seqst", bufs=2) as sq_,
            tc.tile_pool(name="win", bufs=3) as win,
            tc.tile_pool(name="drs", bufs=2, space="DRAM") as drs,
            tc.tile_pool(name="work", bufs=3) as wk,
            tc.tile_pool(name="outp", bufs=2) as op_,
            tc.tile_pool(name="ptr", bufs=2, space="PSUM") as ptr,
            tc.tile_pool(name="pq2", bufs=1, space="PSUM") as pq2,
            tc.tile_pool(name="pP", bufs=2, space="PSUM") as pP,
            tc.tile_pool(name="pO", bufs=2, space="PSUM") as pO,
            tc.tile_pool(name="pS", bufs=1, space="PSUM") as pS,
        ):
            # ---------------- one-time setup (DMA only) ----------------
            id16 = pp.tile([D, D], F16, tag="id16")
            make_identity(nc, id16[:])
            magic_col = pp.tile([D, 1], F32, tag="magic")
            nc.gpsimd.memset(magic_col[:], MAGIC)
            nln2_col = pp.tile([D, 1], F32, tag="nln2")
            nc.gpsimd.memset(nln2_col[:], -LN2)

            omega_t = pp.tile([D, D], F16, tag="omega_t")
            nc.sync.dma_start(out=omega_t[:], in_=om_ext[:, :])
            mask_sb = pp.tile([C, C], F16, tag="mask")
            nc.sync.dma_start(out=mask_sb[:], in_=mask_ext[:, :])
            tb = {}
            for t in _FM_TABLES:
                tb[t] = pp.tile([D, l_eff], F16, tag=t, name=t)
                nc.sync.dma_start(out=tb[t][:], in_=fm_ext[t][:, :])
            for t in _LM_TABLES:
                tb[t] = pp.tile([C, nch * D], F16, tag=t, name=t)
                nc.sync.dma_start(out=tb[t][:], in_=lm_ext[t][:, :])
            for t in _C0_TABLES:
                tb[t] = pp.tile([D, C], F32, tag=t, name=t)
                nc.sync.dma_start(out=tb[t][:], in_=c0_ext[t][:, :])

            # ---------------- per-sequence staging ----------------
            seq_tiles = {}
            ewin = {}
            worder = [(nn, ww) for nn in range(n_seq) for ww in range(NWIN)]

            def issue_loads(gi):
                if gi >= len(worder) or gi in ewin:
                    return
                nn, ww = worder[gi]
                tl = {}
                for nm, ext in (("kw", k_ext), ("qw", q_ext), ("q2w", q2_ext), ("vw", v_ext)):
                    tl[nm] = win.tile([C, CPW * D], F32, tag=nm, name=nm)
                    nc.sync.dma_start(
                        out=tl[nm][:],
                        in_=ext[nn, ww * CPW : (ww + 1) * CPW, :, :].rearrange("c p d -> p c d"),
                    )
                ewin[gi] = tl

            def get_seq(n):
                if n in seq_tiles:
                    return seq_tiles[n]
                t = {}
                for key, shp2 in [
                    ("qt", [D, l_eff]), ("qtc", [D, l_eff]), ("qts", [D, l_eff]),
                    ("kf", [D, l_eff]), ("kcf", [D, l_eff]), ("ksf", [D, l_eff]),
                    ("klm", [C, nch * D]), ("kcl", [C, nch * D]), ("ksl", [C, nch * D]),
                    ("qel", [D, l_eff]), ("nfq", [D, l_eff]),
                ]:
                    t[key] = sq_.tile(shp2, F16, tag=f"{key}_st", name=f"{key}_st")
                t["vw16"] = sq_.tile([C, nch * VST], F16, tag="vw16_st", name="vw16_st")
                seq_tiles[n] = t
                v3 = t["vw16"][:].rearrange("p (c v) -> p c v", v=VST)
                nc.gpsimd.memset(v3[:, :, D : D + 1], 1.0)
                return t

            def emit_E(n, w):
                t = get_seq(n)
                gi = n * NWIN + w
                issue_loads(gi)
                issue_loads(gi + 2)
                tl = ewin.pop(gi)
                kw, qw, q2w, vw = tl["kw"], tl["qw"], tl["q2w"], tl["vw"]
                wsl = bass.ds(w * W, W)
                wdl = bass.ds(w * CPW * D, CPW * D)

                # v cast into per-seq fp16 staging (ones cols pre-set);
                # per-chunk contiguous casts keep the DVE in 2x mode
                for cc in range(CPW):
                    nc.vector.tensor_scalar(
                        t["vw16"][:, bass.ds((w * CPW + cc) * VST, D)],
                        vw[:, bass.ds(cc * D, D)],
                        0.0, None, OP.add,
                    )

                # K path: klm = min(exp(k),1) + max(k,0)
                ek = win.tile([C, CPW * D], F16, tag="ek")
                nc.scalar.activation(ek[:], kw[:], AF.Exp)
                em = win.tile([C, CPW * D], F16, tag="em")
                nc.vector.tensor_scalar(em[:], ek[:], 1.0, None, OP.min)
                rk = win.tile([C, CPW * D], F16, tag="rk")
                nc.vector.tensor_scalar(rk[:], kw[:], 0.0, None, OP.max)
                nc.vector.tensor_tensor(t["klm"][:, wdl], em[:], rk[:], OP.add)

                # kf via per-chunk PE transposes (evacs split vec/scalar)
                for cc in range(CPW):
                    tk = ptr.tile([C, C], F16, tag="tr")
                    nc.tensor.transpose(tk[:], t["klm"][:, bass.ds((w * CPW + cc) * D, D)], id16[:])
                    if cc % 2 == 0:
                        nc.vector.tensor_copy(t["kf"][:, bass.ds(w * W + cc * C, C)], tk[:])
                    else:
                        nc.scalar.activation(t["kf"][:, bass.ds(w * W + cc * C, C)], tk[:], AF.Copy)
                nc.vector.tensor_tensor(t["kcf"][:, wsl], t["kf"][:, wsl], tb["c2t_fm"][:, wsl], OP.mult)

                # Q elu (half): qel = min(exp(q)/2, 1/2) + max(q,0)*0.5
                eq = win.tile([C, CPW * D], F16, tag="eq")
                nc.scalar.activation(eq[:], qw[:], AF.Exp, bias=nln2_col[:, 0:1])
                eh2 = win.tile([C, CPW * D], F16, tag="eh2")
                nc.vector.tensor_scalar(eh2[:], eq[:], 0.5, None, OP.min)
                rqh = win.tile([C, CPW * D], F16, tag="rqh")
                nc.vector.tensor_scalar(rqh[:], qw[:], 0.0, 0.5, OP.max, OP.mult)
                qel_w = win.tile([C, CPW * D], F16, tag="qel_w")
                nc.vector.tensor_tensor(qel_w[:], eh2[:], rqh[:], OP.add)

                # q2 cast to fp16
                q2c = win.tile([C, CPW * D], F16, tag="q2c")
                nc.vector.tensor_scalar(q2c[:], q2w[:], 0.0, None, OP.add)

                # per-chunk fp16 PE transposes: qel evacs on scalar, q2 on vector
                q2f = win.tile([D, W], F16, tag="q2f")
                for cc in range(CPW):
                    lsl = bass.ds(cc * C, C)
                    tq = ptr.tile([C, C], F16, tag="tr")
                    nc.tensor.transpose(tq[:], qel_w[:, bass.ds(cc * D, D)], id16[:])
                    nc.scalar.activation(t["qel"][:, bass.ds(w * W + cc * C, C)], tq[:], AF.Copy)
                    tq2 = ptr.tile([C, C], F16, tag="tr")
                    nc.tensor.transpose(tq2[:], q2c[:, bass.ds(cc * D, D)], id16[:])
                    nc.vector.tensor_copy(q2f[:, lsl], tq2[:])

                # q2 projection (fp16) + magic range reduction -> nfq in [-.5,.5]
                yp = pq2.tile([D, W], F32, tag="q2p")
                nc.tensor.matmul(yp[:], omega_t[:], q2f[:], start=True, stop=True)
                kq = win.tile([D, W], F32, tag="kq")
                nc.scalar.activation(kq[:], yp[:], AF.Identity, bias=magic_col[:, 0:1])
                nc.vector.scalar_tensor_tensor(t["nfq"][:, wsl], kq[:], MAGIC, yp[:], OP.subtract, OP.subtract)

            def emit_E_tail(n):
                t = get_seq(n)
                nc.gpsimd.tensor_tensor(t["kcl"][:], t["klm"][:], tb["c2t_lm"][:], OP.mult)
                nc.gpsimd.tensor_tensor(t["ksl"][:], t["klm"][:], tb["s2t_lm"][:], OP.mult)
                nc.gpsimd.tensor_tensor(t["ksf"][:], t["kf"][:], tb["s2t_fm"][:], OP.mult)

            def emit_T(n, w):
                t = get_seq(n)
                wsl = bass.ds(w * W, W)
                sqw = win.tile([D, W], F16, tag="sqw")
                nc.scalar.activation(sqw[:], t["nfq"][:, wsl], AF.Sin, scale=-TWO_PI)
                s2w = win.tile([D, W], F16, tag="s2w")
                nc.scalar.activation(s2w[:], sqw[:], AF.Square)
                nc.vector.tensor_tensor(t["qt"][:, wsl], s2w[:], t["qel"][:, wsl], OP.mult)

            def emit_qtcs(n, half):
                t = get_seq(n)
                hl = bass.ds(half * (L // 2), L // 2)
                nc.gpsimd.tensor_tensor(t["qtc"][:, hl], t["qt"][:, hl], tb["c2n_fm"][:, hl], OP.mult)
                nc.gpsimd.tensor_tensor(t["qts"][:, hl], t["qt"][:, hl], tb["s2n_fm"][:, hl], OP.mult)

            scan_state = {}

            def emit_scan(n, chunks):
                t = seq_tiles[n]
                st = scan_state.setdefault(n, {"st_ps": None, "sst": None, "ob4": None})
                for c in chunks:
                    first, last = c == 0, c == nch - 1
                    cc = c % CPW
                    sl = bass.ts(c, C)
                    dsl = bass.ts(c, D)
                    vp = t["vw16"][:, bass.ds(c * VST, DV1)]

                    p_ps = pP.tile([C, C], F32, tag="P")
                    if first:
                        st["st_ps"] = pS.tile([D, 3 * SW], F32, tag="st", name="st_ps")
                        qa = wk.tile([D, C], F32, tag="qa")
                        nc.gpsimd.tensor_tensor(qa[:], t["qt"][:, 0:C], tb["qs2_0"][:], OP.mult)
                        qb = wk.tile([D, C], F32, tag="qb")
                        nc.gpsimd.tensor_tensor(qb[:], t["qt"][:, 0:C], tb["qc2_0"][:], OP.mult)
                        qc = wk.tile([D, C], F32, tag="qc")
                        nc.gpsimd.tensor_tensor(qc[:], t["qt"][:, 0:C], tb["qsc_0"][:], OP.mult)
                        ka = wk.tile([D, C], F32, tag="ka")
                        nc.gpsimd.tensor_tensor(ka[:], t["kf"][:, 0:C], tb["kc2_0"][:], OP.mult)
                        kb = wk.tile([D, C], F32, tag="kb")
                        nc.gpsimd.tensor_tensor(kb[:], t["kf"][:, 0:C], tb["ks2_0"][:], OP.mult)
                        kc = wk.tile([D, C], F32, tag="kc")
                        nc.gpsimd.tensor_tensor(kc[:], t["kf"][:, 0:C], tb["ksc_0"][:], OP.mult)
                        nc.tensor.matmul(p_ps[:], ka[:], qa[:], start=True, stop=False)
                        nc.tensor.matmul(p_ps[:], kb[:], qb[:], start=False, stop=False)
                        nc.tensor.matmul(p_ps[:], kc[:], qc[:], start=False, stop=True)
                    else:
                        nc.tensor.matmul(p_ps[:], t["kf"][:, sl], t["qt"][:, sl], start=True, stop=False)
                        nc.tensor.matmul(p_ps[:], t["kcf"][:, sl], t["qtc"][:, sl], start=False, stop=False)
                        nc.tensor.matmul(p_ps[:], t["ksf"][:, sl], t["qts"][:, sl], start=False, stop=True)

                    p_sb = wk.tile([C, C], F16, tag="p_sb")
                    nc.vector.tensor_tensor(p_sb[:], p_ps[:], mask_sb[:], OP.mult)

                    o_ps = pO.tile([C, DV1], F32, tag="O")
                    nc.tensor.matmul(o_ps[:], p_sb[:], vp, start=True, stop=first)
                    if not first:
                        sst = st["sst"]
                        nc.tensor.matmul(o_ps[:], t["qt"][:, sl], sst[:, 0:DV1], start=False, stop=False)
                        nc.tensor.matmul(o_ps[:], t["qtc"][:, sl], sst[:, DV1 : 2 * DV1], start=False, stop=False)
                        nc.tensor.matmul(o_ps[:], t["qts"][:, sl], sst[:, 2 * DV1 : 3 * DV1], start=False, stop=True)

                    if not last:
                        sp = st["st_ps"]
                        nc.tensor.matmul(sp[:, 0:DV1], t["klm"][:, dsl], vp, start=first, stop=True, skip_group_check=not first)
                        nc.tensor.matmul(sp[:, SW : SW + DV1], t["kcl"][:, dsl], vp, start=False, stop=True, skip_group_check=True)
                        nc.tensor.matmul(sp[:, 2 * SW : 2 * SW + DV1], t["ksl"][:, dsl], vp, start=False, stop=True, skip_group_check=True)
                        sst = wk.tile([D, 3 * DV1], F16, tag="sst")
                        nc.scalar.activation(
                            sst[:].rearrange("p (g x) -> p g x", x=DV1),
                            sp[:].rearrange("p (g x) -> p g x", x=SW)[:, :, 0:DV1],
                            AF.Copy,
                        )
                        st["sst"] = sst

                    if cc == 0:
                        st["ob4"] = op_.tile([C, CPW * DV1], F16, tag="ob4", name="ob4")
                    nc.scalar.activation(st["ob4"][:, bass.ds(cc * DV1, DV1)], o_ps[:, 0:DV1], AF.Copy, scale=0.0625)
                    if cc == CPW - 1:
                        nc.gpsimd.dma_start(out=out_ext[n, c // CPW, :, :], in_=st["ob4"][:])

            # ---------------- emission schedule ----------------
            def t_phase(n):
                emit_T(n, 0)
                emit_T(n, 1)
                emit_qtcs(n, 0)
                emit_scan(n, [0, 1, 2])
                emit_T(n, 2)
                emit_scan(n, [3, 4, 5])
                emit_T(n, 3)
                emit_qtcs(n, 1)
                emit_scan(n, [6, 7, 8])
                emit_scan(n, [9, 10, 11])

            issue_loads(0)
            issue_loads(1)
            for w in range(NWIN):
                emit_E(0, w)
            emit_E_tail(0)
            t_phase(0)
            for n in range(1, n_seq):
                for w in range(NWIN):
                    emit_scan(n - 1, [12 + w])
                    emit_E(n, w)
                emit_E_tail(n)
                del seq_tiles[n - 1]
                t_phase(n)
            emit_scan(n_seq - 1, list(range(12, nch)))

    nc.finalize()
    return nc


def _host_tables(om_h):
    """Trig tables for one head from omega [D, D] (float64 math)."""
    om64 = om_h.astype(np.float64)
    w = om64.sum(axis=0)  # w[j] = sum_i omega[i, j]
    t = np.outer(w, np.arange(L, dtype=np.float64) / L)  # [D, L]
    s, c = np.sin(t), np.cos(t)
    s0, c0 = s[:, :C], c[:, :C]
    c2, s2 = np.cos(2.0 * t), np.sin(2.0 * t)
    lm = lambda x: np.ascontiguousarray(
        x.reshape(D, NCH, C).transpose(2, 1, 0).reshape(C, NCH * D)
    )
    return {
        "omega16": (om64 / TWO_PI).astype(np.float16),
        "qs2_0": (s0**2).astype(np.float32),
        "qc2_0": (c0**2).astype(np.float32),
        "qsc_0": (-2.0 * s0 * c0).astype(np.float32),
        "kc2_0": (2.0 * c0**2).astype(np.float32),
        "ks2_0": (2.0 * s0**2).astype(np.float32),
        "ksc_0": (2.0 * s0 * c0).astype(np.float32),
        "c2t_fm": c2.astype(np.float16),
        "s2t_fm": s2.astype(np.float16),
        "c2n_fm": (-c2).astype(np.float16),
        "s2n_fm": (-s2).astype(np.float16),
        "c2t_lm": lm(c2).astype(np.float16),
        "s2t_lm": lm(s2).astype(np.float16),
    }


def _host_inputs(inputs, n_seq=N, nch=NCH):
    l_eff = nch * C
    q = np.ascontiguousarray(inputs["queries"], dtype=np.float32)
    q2 = np.ascontiguousarray(inputs["q2"], dtype=np.float32)
    k = np.ascontiguousarray(inputs["keys"], dtype=np.float32)
    v = np.ascontiguousarray(inputs["values"], dtype=np.float32)
    om = np.ascontiguousarray(inputs["omega"], dtype=np.float32)

    mask = np.triu(np.ones((C, C), dtype=np.float16))

    def shp(x, h):
        return np.ascontiguousarray(x[:n_seq, :l_eff, h, :]).reshape(n_seq, nch, C, D)

    in_maps = []
    for h in range(om.shape[0] if om.ndim == 3 else H):
        m = {
            "queries": shp(q, h),
            "q2": shp(q2, h),
            "keys": shp(k, h),
            "values": shp(v, h),
            "mask": mask,
        }
        m.update(_host_tables(om[h]))
        in_maps.append(m)
    return in_maps


def _run(inputs, trace=False):
    if "nc" not in _CACHE:
        _CACHE["nc"] = build_nc()
    nc = _CACHE["nc"]
    in_maps = _host_inputs(inputs)
    res = run_bass_kernel_spmd(nc, in_maps, core_ids=list(range(H)), trace=trace)
    outs = []
    for hh in range(H):
        o = res.results[hh]["out"].reshape(N, NWIN, C, CPW, DV1).astype(np.float32)
        num, z = o[..., :D], o[..., D]
        o = num / (z + EPS * 0.0625)[..., None]
        outs.append(o.transpose(0, 1, 3, 2, 4).reshape(N, L, D))
    full = np.stack(outs, axis=2)
    return full.astype(np.float32), res


def kernel(**inputs):
    out, _ = _run(inputs, trace=False)
    return out


# revision 18
# speedup vs baseline: 1.0386x; 1.0386x over previous
"""v6: host-precomputed trig tables + vector-lean feature pipeline.

Structure (per core = one head, 4 sequences):
  - All trig tables (double-angle FM/LM fp16, chunk-0 exact fp32, omega/2pi
    fp16) are computed host-side from omega and DMA'd in; no on-device
    table generation.
  - Feature phase split per sequence into an Exp pass (E) and Trig pass (T)
    so the scalar engine loads each activation table set once per sequence.
  - elu1 fused as min(exp(x),1)+max(x,0) via TS(min)+TT(add); exp reads the
    raw fp32 window directly (inf clamps through min).
  - fp32->fp16 casts via tensor_scalar (tensor_copy falls to 1x mode).
  - GpSimd: sequence-wide TTs for ksl/ksf/qts/qtc + chunk-0 branch TTs.
  - Scan: 3 branch P matmuls -> masked p_sb -> intra/inter O matmuls; 3
    states share one PSUM bank with a single fused fp16 evacuation.
  - Scan of seq n-1 interleaved 2 chunks per feature sub-slot of seq n.
"""

import math

import numpy as np

import concourse.bass as bass
import concourse.tile as tile
from concourse import bacc, mybir
from concourse.bass_utils import run_bass_kernel_spmd
from concourse.masks import make_identity

F32 = mybir.dt.float32
F16 = mybir.dt.float16
AF = mybir.ActivationFunctionType
OP = mybir.AluOpType

N, L, H, D = 4, 2048, 8, 128
C = 128
NCH = L // C
DV1 = D + 1
VST = 130  # per-chunk slot width in v staging (4B-aligned stride)
SW = 130  # state slice stride in PSUM cols (8B aligned)
TWO_PI = 2.0 * math.pi
MAGIC = float(np.float32(1.5 * 2**23))
LN2 = float(np.log(2.0))
EPS = 1e-6
W = 512
CPW = W // C
NWIN = L // W

_CACHE = {}

_FM_TABLES = ["c2t_fm", "s2t_fm", "c2n_fm", "s2n_fm"]
_LM_TABLES = ["c2t_lm", "s2t_lm"]
_C0_TABLES = ["qs2_0", "qc2_0", "qsc_0", "kc2_0", "ks2_0", "ksc_0"]


def build_nc(n_seq=N, nch=NCH):
    l_eff = nch * C
    nc = bacc.Bacc(None, target_bir_lowering=False, debug=False)

    q_ext = nc.declare_dram_parameter("queries", [n_seq, nch, C, D], F32, isOutput=False)
    q2_ext = nc.declare_dram_parameter("q2", [n_seq, nch, C, D], F32, isOutput=False)
    k_ext = nc.declare_dram_parameter("keys", [n_seq, nch, C, D], F32, isOutput=False)
    v_ext = nc.declare_dram_parameter("values", [n_seq, nch, C, D], F32, isOutput=False)
    om_ext = nc.declare_dram_parameter("omega16", [D, D], F16, isOutput=False)
    mask_ext = nc.declare_dram_parameter("mask", [C, C], F16, isOutput=False)
    fm_ext = {t: nc.declare_dram_parameter(t, [D, l_eff], F16, isOutput=False) for t in _FM_TABLES}
    lm_ext = {t: nc.declare_dram_parameter(t, [C, nch * D], F16, isOutput=False) for t in _LM_TABLES}
    c0_ext = {t: nc.declare_dram_parameter(t, [D, C], F32, isOutput=False) for t in _C0_TABLES}
    out_ext = nc.declare_dram_parameter("out", [n_seq, NWIN, C, CPW * DV1], F16, isOutput=True)

    with tile.TileContext(nc) as tc:
        with (
            tc.tile_pool(name="persist", bufs=1) as pp,
            tc.tile_pool(name="seqst", bufs=2) as sq_,
            tc.tile_pool(name="win", bufs=3) as win,
            tc.tile_pool(name="drs", bufs=2, space="DRAM") as drs,
            tc.tile_pool(name="work", bufs=3) as wk,
            tc.tile_pool(name="outp", bufs=2) as op_,
            tc.tile_pool(name="ptf", bufs=2, space="PSUM") as ptf,
            tc.tile_pool(name="pq2", bufs=1, space="PSUM") as pq2,
            tc.tile_pool(name="pP", bufs=2, space="PSUM") as pP,
            tc.tile_pool(name="pO", bufs=2, space="PSUM") as pO,
            tc.tile_pool(name="pS", bufs=1, space="PSUM") as pS,
        ):
            # ---------------- one-time setup (DMA only) ----------------
            id16 = pp.tile([D, D], F16, tag="id16")
            make_identity(nc, id16[:])
            magic_col = pp.tile([D, 1], F32, tag="magic")
            nc.gpsimd.memset(magic_col[:], MAGIC)
            nln2_col = pp.tile([D, 1], F32, tag="nln2")
            nc.gpsimd.memset(nln2_col[:], -LN2)

            omega_t = pp.tile([D, D], F16, tag="omega_t")
            nc.sync.dma_start(out=omega_t[:], in_=om_ext[:, :])
            mask_sb = pp.tile([C, C], F16, tag="mask")
            nc.sync.dma_start(out=mask_sb[:], in_=mask_ext[:, :])
            tb = {}
            for t in _FM_TABLES:
                tb[t] = pp.tile([D, l_eff], F16, tag=t, name=t)
                nc.sync.dma_start(out=tb[t][:], in_=fm_ext[t][:, :])
            for t in _LM_TABLES:
                tb[t] = pp.tile([C, nch * D], F16, tag=t, name=t)
                nc.sync.dma_start(out=tb[t][:], in_=lm_ext[t][:, :])
            for t in _C0_TABLES:
                tb[t] = pp.tile([D, C], F32, tag=t, name=t)
                nc.sync.dma_start(out=tb[t][:], in_=c0_ext[t][:, :])

            # ---------------- per-sequence staging ----------------
            seq_tiles = {}
            ewin = {}
            worder = [(nn, ww) for nn in range(n_seq) for ww in range(NWIN)]

            def issue_loads(gi):
                if gi >= len(worder) or gi in ewin:
                    return
                nn, ww = worder[gi]
                tl = {}
                for nm, ext in (("kw", k_ext), ("qw", q_ext), ("q2w", q2_ext), ("vw", v_ext)):
                    tl[nm] = win.tile([C, CPW * D], F32, tag=nm, name=nm)
                    nc.sync.dma_start(
                        out=tl[nm][:],
                        in_=ext[nn, ww * CPW : (ww + 1) * CPW, :, :].rearrange("c p d -> p c d"),
                    )
                ewin[gi] = tl

            def get_seq(n):
                if n in seq_tiles:
                    return seq_tiles[n]
                t = {}
                for key, shp2 in [
                    ("qt", [D, l_eff]), ("qtc", [D, l_eff]), ("qts", [D, l_eff]),
                    ("kf", [D, l_eff]), ("kcf", [D, l_eff]), ("ksf", [D, l_eff]),
                    ("klm", [C, nch * D]), ("kcl", [C, nch * D]), ("ksl", [C, nch * D]),
                    ("qel", [D, l_eff]), ("nfq", [D, l_eff]),
                ]:
                    t[key] = sq_.tile(shp2, F16, tag=f"{key}_st", name=f"{key}_st")
                t["vw16"] = sq_.tile([C, nch * VST], F16, tag="vw16_st", name="vw16_st")
                seq_tiles[n] = t
                v3 = t["vw16"][:].rearrange("p (c v) -> p c v", v=VST)
                nc.gpsimd.memset(v3[:, :, D : D + 1], 1.0)
                return t

            def emit_E(n, w):
                t = get_seq(n)
                gi = n * NWIN + w
                issue_loads(gi)
                issue_loads(gi + 2)
                tl = ewin.pop(gi)
                kw, qw, q2w, vw = tl["kw"], tl["qw"], tl["q2w"], tl["vw"]
                wsl = bass.ds(w * W, W)
                wdl = bass.ds(w * CPW * D, CPW * D)

                # v cast into per-seq fp16 staging (ones cols pre-set);
                # per-chunk contiguous casts keep the DVE in 2x mode
                for cc in range(CPW):
                    nc.vector.tensor_scalar(
                        t["vw16"][:, bass.ds((w * CPW + cc) * VST, D)],
                        vw[:, bass.ds(cc * D, D)],
                        0.0, None, OP.add,
                    )

                # K path: klm = min(exp(k),1) + max(k,0)
                ek = win.tile([C, CPW * D], F16, tag="ek")
                nc.scalar.activation(ek[:], kw[:], AF.Exp)
                em = win.tile([C, CPW * D], F16, tag="em")
                nc.vector.tensor_scalar(em[:], ek[:], 1.0, None, OP.min)
                rk = win.tile([C, CPW * D], F16, tag="rk")
                nc.vector.tensor_scalar(rk[:], kw[:], 0.0, None, OP.max)
                nc.vector.tensor_tensor(t["klm"][:, wdl], em[:], rk[:], OP.add)

                # kf via batched PE transposes into one PSUM bank, single evac
                ptk = ptf.tile([D, W], F16, tag="ptf", name="ptk")
                for cc in range(CPW):
                    nc.tensor.transpose(ptk[:, bass.ds(cc * C, C)], t["klm"][:, bass.ds((w * CPW + cc) * D, D)], id16[:])
                nc.vector.tensor_copy(t["kf"][:, wsl], ptk[:])
                nc.vector.tensor_tensor(t["kcf"][:, wsl], t["kf"][:, wsl], tb["c2t_fm"][:, wsl], OP.mult)

                # Q elu (half): qel = min(exp(q)/2, 1/2) + max(q,0)*0.5
                eq = win.tile([C, CPW * D], F16, tag="eq")
                nc.scalar.activation(eq[:], qw[:], AF.Exp, bias=nln2_col[:, 0:1])
                eh2 = win.tile([C, CPW * D], F16, tag="eh2")
                nc.vector.tensor_scalar(eh2[:], eq[:], 0.5, None, OP.min)
                rqh = win.tile([C, CPW * D], F16, tag="rqh")
                nc.vector.tensor_scalar(rqh[:], qw[:], 0.0, 0.5, OP.max, OP.mult)
                qel_w = win.tile([C, CPW * D], F16, tag="qel_w")
                nc.vector.tensor_tensor(qel_w[:], eh2[:], rqh[:], OP.add)

                # q2 cast to fp16
                q2c = win.tile([C, CPW * D], F16, tag="q2c")
                nc.vector.tensor_scalar(q2c[:], q2w[:], 0.0, None, OP.add)

                # batched fp16 PE transposes; one fused evac per tensor
                ptq = ptf.tile([D, W], F16, tag="ptf", name="ptq")
                for cc in range(CPW):
                    nc.tensor.transpose(ptq[:, bass.ds(cc * C, C)], qel_w[:, bass.ds(cc * D, D)], id16[:])
                nc.scalar.activation(t["qel"][:, wsl], ptq[:], AF.Copy)
                ptq2 = ptf.tile([D, W], F16, tag="ptf", name="ptq2")
                q2f = win.tile([D, W], F16, tag="q2f")
                for cc in range(CPW):
                    nc.tensor.transpose(ptq2[:, bass.ds(cc * C, C)], q2c[:, bass.ds(cc * D, D)], id16[:])
                nc.vector.tensor_copy(q2f[:], ptq2[:])

                # q2 projection (fp16) + magic range reduction -> nfq in [-.5,.5]
                yp = pq2.tile([D, W], F32, tag="q2p")
                nc.tensor.matmul(yp[:], omega_t[:], q2f[:], start=True, stop=True)
                kq = win.tile([D, W], F32, tag="kq")
                nc.scalar.activation(kq[:], yp[:], AF.Identity, bias=magic_col[:, 0:1])
                nc.vector.scalar_tensor_tensor(t["nfq"][:, wsl], kq[:], MAGIC, yp[:], OP.subtract, OP.subtract)

            def emit_E_tail(n):
                t = get_seq(n)
                nc.gpsimd.tensor_tensor(t["kcl"][:], t["klm"][:], tb["c2t_lm"][:], OP.mult)
                nc.gpsimd.tensor_tensor(t["ksl"][:], t["klm"][:], tb["s2t_lm"][:], OP.mult)
                nc.gpsimd.tensor_tensor(t["ksf"][:], t["kf"][:], tb["s2t_fm"][:], OP.mult)

            def emit_T(n, w):
                t = get_seq(n)
                wsl = bass.ds(w * W, W)
                sqw = win.tile([D, W], F16, tag="sqw")
                nc.scalar.activation(sqw[:], t["nfq"][:, wsl], AF.Sin, scale=-TWO_PI)
                s2w = win.tile([D, W], F16, tag="s2w")
                nc.scalar.activation(s2w[:], sqw[:], AF.Square)
                nc.vector.tensor_tensor(t["qt"][:, wsl], s2w[:], t["qel"][:, wsl], OP.mult)

            def emit_qtcs(n, half):
                t = get_seq(n)
                hl = bass.ds(half * (L // 2), L // 2)
                nc.gpsimd.tensor_tensor(t["qtc"][:, hl], t["qt"][:, hl], tb["c2n_fm"][:, hl], OP.mult)
                nc.gpsimd.tensor_tensor(t["qts"][:, hl], t["qt"][:, hl], tb["s2n_fm"][:, hl], OP.mult)

            scan_state = {}

            def emit_scan(n, chunks):
                t = seq_tiles[n]
                st = scan_state.setdefault(n, {"st_ps": None, "sst": None, "ob4": None})
                for c in chunks:
                    first, last = c == 0, c == nch - 1
                    cc = c % CPW
                    sl = bass.ts(c, C)
                    dsl = bass.ts(c, D)
                    vp = t["vw16"][:, bass.ds(c * VST, DV1)]

                    p_ps = pP.tile([C, C], F32, tag="P")
                    if first:
                        st["st_ps"] = pS.tile([D, 3 * SW], F32, tag="st", name="st_ps")
                        qa = wk.tile([D, C], F32, tag="qa")
                        nc.gpsimd.tensor_tensor(qa[:], t["qt"][:, 0:C], tb["qs2_0"][:], OP.mult)
                        qb = wk.tile([D, C], F32, tag="qb")
                        nc.gpsimd.tensor_tensor(qb[:], t["qt"][:, 0:C], tb["qc2_0"][:], OP.mult)
                        qc = wk.tile([D, C], F32, tag="qc")
                        nc.gpsimd.tensor_tensor(qc[:], t["qt"][:, 0:C], tb["qsc_0"][:], OP.mult)
                        ka = wk.tile([D, C], F32, tag="ka")
                        nc.gpsimd.tensor_tensor(ka[:], t["kf"][:, 0:C], tb["kc2_0"][:], OP.mult)
                        kb = wk.tile([D, C], F32, tag="kb")
                        nc.gpsimd.tensor_tensor(kb[:], t["kf"][:, 0:C], tb["ks2_0"][:], OP.mult)
                        kc = wk.tile([D, C], F32, tag="kc")
                        nc.gpsimd.tensor_tensor(kc[:], t["kf"][:, 0:C], tb["ksc_0"][:], OP.mult)
                        nc.tensor.matmul(p_ps[:], ka[:], qa[:], start=True, stop=False)
                        nc.tensor.matmul(p_ps[:], kb[:], qb[:], start=False, stop=False)
                        nc.tensor.matmul(p_ps[:], kc[:], qc[:], start=False, stop=True)
                    else:
                        nc.tensor.matmul(p_ps[:], t["kf"][:, sl], t["qt"][:, sl], start=True, stop=False)
                        nc.tensor.matmul(p_ps[:], t["kcf"][:, sl], t["qtc"][:, sl], start=False, stop=False)
                        nc.tensor.matmul(p_ps[:], t["ksf"][:, sl], t["qts"][:, sl], start=False, stop=True)

                    p_sb = wk.tile([C, C], F16, tag="p_sb")
                    nc.vector.tensor_tensor(p_sb[:], p_ps[:], mask_sb[:], OP.mult)

                    o_ps = pO.tile([C, DV1], F32, tag="O")
                    nc.tensor.matmul(o_ps[:], p_sb[:], vp, start=True, stop=first)
                    if not first:
                        sst = st["sst"]
                        nc.tensor.matmul(o_ps[:], t["qt"][:, sl], sst[:, 0:DV1], start=False, stop=False)
                        nc.tensor.matmul(o_ps[:], t["qtc"][:, sl], sst[:, SW : SW + DV1], start=False, stop=False)
                        nc.tensor.matmul(o_ps[:], t["qts"][:, sl], sst[:, 2 * SW : 2 * SW + DV1], start=False, stop=True)

                    if not last:
                        sp = st["st_ps"]
                        nc.tensor.matmul(sp[:, 0:DV1], t["klm"][:, dsl], vp, start=first, stop=True, skip_group_check=not first)
                        nc.tensor.matmul(sp[:, SW : SW + DV1], t["kcl"][:, dsl], vp, start=False, stop=True, skip_group_check=True)
                        nc.tensor.matmul(sp[:, 2 * SW : 2 * SW + DV1], t["ksl"][:, dsl], vp, start=False, stop=True, skip_group_check=True)
                        sst = wk.tile([D, 3 * SW], F16, tag="sst")
                        nc.scalar.activation(sst[:], sp[:], AF.Copy)
                        st["sst"] = sst

                    if cc == 0:
                        st["ob4"] = op_.tile([C, CPW * DV1], F16, tag="ob4", name="ob4")
                    nc.scalar.activation(st["ob4"][:, bass.ds(cc * DV1, DV1)], o_ps[:, 0:DV1], AF.Copy, scale=0.0625)
                    if cc == CPW - 1:
                        nc.gpsimd.dma_start(out=out_ext[n, c // CPW, :, :], in_=st["ob4"][:])

            # ---------------- emission schedule ----------------
            def t_phase(n):
                emit_T(n, 0)
                emit_T(n, 1)
                emit_qtcs(n, 0)
                emit_scan(n, [0, 1, 2])
                emit_T(n, 2)
                emit_scan(n, [3, 4, 5])
                emit_T(n, 3)
                emit_qtcs(n, 1)
                emit_scan(n, [6, 7, 8])
                emit_scan(n, [9, 10, 11])

            issue_loads(0)
            issue_loads(1)
            for w in range(NWIN):
                emit_E(0, w)
            emit_E_tail(0)
            t_phase(0)
            for n in range(1, n_seq):
                for w in range(NWIN):
                    emit_scan(n - 1, [12 + w])
                    emit_E(n, w)
                emit_E_tail(n)
                del seq_tiles[n - 1]
                t_phase(n)
            emit_scan(n_seq - 1, list(range(12, nch)))

    nc.finalize()
    return nc


def _host_tables(om_h):
    """Trig tables for one head from omega [D, D] (float64 math)."""
    om64 = om_h.astype(np.float64)
    w = om64.sum(axis=0)  # w[j] = sum_i omega[i, j]
    t = np.outer(w, np.arange(L, dtype=np.float64) / L)  # [D, L]
    s, c = np.sin(t), np.cos(t)
    s0, c0 = s[:, :C], c[:, :C]
    c2, s2 = np.cos(2.0 * t), np.sin(2.0 * t)
    lm = lambda x: np.ascontiguousarray(
        x.reshape(D, NCH, C).transpose(2, 1, 0).reshape(C, NCH * D)
    )
    return {
        "omega16": (om64 / TWO_PI).astype(np.float16),
        "qs2_0": (s0**2).astype(np.float32),
        "qc2_0": (c0**2).astype(np.float32),
        "qsc_0": (-2.0 * s0 * c0).astype(np.float32),
        "kc2_0": (2.0 * c0**2).astype(np.float32),
        "ks2_0": (2.0 * s0**2).astype(np.float32),
        "ksc_0": (2.0 * s0 * c0).astype(np.float32),
        "c2t_fm": c2.astype(np.float16),
        "s2t_fm": s2.astype(np.float16),
        "c2n_fm": (-c2).astype(np.float16),
        "s2n_fm": (-s2).astype(np.float16),
        "c2t_lm": lm(c2).astype(np.float16),
        "s2t_lm": lm(s2).astype(np.float16),
    }


def _host_inputs(inputs, n_seq=N, nch=NCH):
    l_eff = nch * C
    q = np.ascontiguousarray(inputs["queries"], dtype=np.float32)
    q2 = np.ascontiguousarray(inputs["q2"], dtype=np.float32)
    k = np.ascontiguousarray(inputs["keys"], dtype=np.float32)
    v = np.ascontiguousarray(inputs["values"], dtype=np.float32)
    om = np.ascontiguousarray(inputs["omega"], dtype=np.float32)

    mask = np.triu(np.ones((C, C), dtype=np.float16))

    def shp(x, h):
        return np.ascontiguousarray(x[:n_seq, :l_eff, h, :]).reshape(n_seq, nch, C, D)

    in_maps = []
    for h in range(om.shape[0] if om.ndim == 3 else H):
        m = {
            "queries": shp(q, h),
            "q2": shp(q2, h),
            "keys": shp(k, h),
            "values": shp(v, h),
            "mask": mask,
        }
        m.update(_host_tables(om[h]))
        in_maps.append(m)
    return in_maps


def _run(inputs, trace=False):
    if "nc" not in _CACHE:
        _CACHE["nc"] = build_nc()
    nc = _CACHE["nc"]
    in_maps = _host_inputs(inputs)
    res = run_bass_kernel_spmd(nc, in_maps, core_ids=list(range(H)), trace=trace)
    outs = []
    for hh in range(H):
        o = res.results[hh]["out"].reshape(N, NWIN, C, CPW, DV1).astype(np.float32)
        num, z = o[..., :D], o[..., D]
        o = num / (z + EPS * 0.0625)[..., None]
        outs.append(o.transpose(0, 1, 3, 2, 4).reshape(N, L, D))
    full = np.stack(outs, axis=2)
    return full.astype(np.float32), res


def kernel(**inputs):
    out, _ = _run(inputs, trace=False)
    return out


# revision 19
# speedup vs baseline: 1.1809x; 1.1370x over previous
"""v6: host-precomputed trig tables + vector-lean feature pipeline.

Structure (per core = one head, 4 sequences):
  - All trig tables (double-angle FM/LM fp16, chunk-0 exact fp32, omega/2pi
    fp16) are computed host-side from omega and DMA'd in; no on-device
    table generation.
  - Feature phase split per sequence into an Exp pass (E) and Trig pass (T)
    so the scalar engine loads each activation table set once per sequence.
  - elu1 fused as min(exp(x),1)+max(x,0) via TS(min)+TT(add); exp reads the
    raw fp32 window directly (inf clamps through min).
  - fp32->fp16 casts via tensor_scalar (tensor_copy falls to 1x mode).
  - GpSimd: sequence-wide TTs for ksl/ksf/qts/qtc + chunk-0 branch TTs.
  - Scan: 3 branch P matmuls -> masked p_sb -> intra/inter O matmuls; 3
    states share one PSUM bank with a single fused fp16 evacuation.
  - Scan of seq n-1 interleaved 2 chunks per feature sub-slot of seq n.
"""

import math

import numpy as np

import concourse.bass as bass
import concourse.tile as tile
from concourse import bacc, mybir
from concourse.bass_utils import run_bass_kernel_spmd
from concourse.masks import make_identity

F32 = mybir.dt.float32
F16 = mybir.dt.float16
AF = mybir.ActivationFunctionType
OP = mybir.AluOpType

N, L, H, D = 4, 2048, 8, 128
C = 128
NCH = L // C
DV1 = D + 1
VST = 130  # per-chunk slot width in v staging (4B-aligned stride)
SW = 130  # state slice stride in PSUM cols (8B aligned)
TWO_PI = 2.0 * math.pi
MAGIC = float(np.float32(1.5 * 2**23))
LN2 = float(np.log(2.0))
EPS = 1e-6
W = 512
CPW = W // C
NWIN = L // W

_CACHE = {}

_FM_TABLES = ["c2t_fm", "s2t_fm", "c2n_fm", "s2n_fm"]
_LM_TABLES = ["c2t_lm", "s2t_lm"]
_C0_TABLES = ["qs2_0", "qc2_0", "qsc_0", "kc2_0", "ks2_0", "ksc_0"]


def build_nc(n_seq=N, nch=NCH):
    l_eff = nch * C
    nc = bacc.Bacc(None, target_bir_lowering=False, debug=False)

    q_ext = nc.declare_dram_parameter("queries", [n_seq, nch, C, D], F32, isOutput=False)
    q2_ext = nc.declare_dram_parameter("q2", [n_seq, nch, C, D], F32, isOutput=False)
    k_ext = nc.declare_dram_parameter("keys", [n_seq, nch, C, D], F32, isOutput=False)
    v_ext = nc.declare_dram_parameter("values", [n_seq, nch, C, D], F32, isOutput=False)
    om_ext = nc.declare_dram_parameter("omega16", [D, D], F16, isOutput=False)
    mask_ext = nc.declare_dram_parameter("mask", [C, C], F16, isOutput=False)
    fm_ext = {t: nc.declare_dram_parameter(t, [D, l_eff], F16, isOutput=False) for t in _FM_TABLES}
    lm_ext = {t: nc.declare_dram_parameter(t, [C, nch * D], F16, isOutput=False) for t in _LM_TABLES}
    c0_ext = {t: nc.declare_dram_parameter(t, [D, C], F32, isOutput=False) for t in _C0_TABLES}
    out_ext = nc.declare_dram_parameter("out", [n_seq, NWIN, C, CPW * DV1], F16, isOutput=True)

    with tile.TileContext(nc) as tc:
        with (
            tc.tile_pool(name="persist", bufs=1) as pp,
            tc.tile_pool(name="seqst", bufs=2) as sq_,
            tc.tile_pool(name="win", bufs=3) as win,
            tc.tile_pool(name="drs", bufs=2, space="DRAM") as drs,
            tc.tile_pool(name="work", bufs=3) as wk,
            tc.tile_pool(name="outp", bufs=2) as op_,
            tc.tile_pool(name="ptf", bufs=2, space="PSUM") as ptf,
            tc.tile_pool(name="pq2", bufs=1, space="PSUM") as pq2,
            tc.tile_pool(name="pP", bufs=2, space="PSUM") as pP,
            tc.tile_pool(name="pO", bufs=2, space="PSUM") as pO,
            tc.tile_pool(name="pS", bufs=1, space="PSUM") as pS,
        ):
            # ---------------- one-time setup (DMA only) ----------------
            id16 = pp.tile([D, D], F16, tag="id16")
            make_identity(nc, id16[:])
            magic_col = pp.tile([D, 1], F32, tag="magic")
            nc.gpsimd.memset(magic_col[:], MAGIC)
            nln2_col = pp.tile([D, 1], F32, tag="nln2")
            nc.gpsimd.memset(nln2_col[:], -LN2)

            omega_t = pp.tile([D, D], F16, tag="omega_t")
            nc.sync.dma_start(out=omega_t[:], in_=om_ext[:, :])
            mask_sb = pp.tile([C, C], F16, tag="mask")
            nc.sync.dma_start(out=mask_sb[:], in_=mask_ext[:, :])
            tb = {}
            for t in _FM_TABLES:
                tb[t] = pp.tile([D, l_eff], F16, tag=t, name=t)
            for t in _LM_TABLES:
                tb[t] = pp.tile([C, nch * D], F16, tag=t, name=t)
            for t in _C0_TABLES:
                tb[t] = pp.tile([D, C], F32, tag=t, name=t)
            for t in ["c2t_fm", "c2t_lm", "s2t_lm", "s2t_fm", "c2n_fm", "s2n_fm"]:
                ext = fm_ext[t] if t in fm_ext else lm_ext[t]
                nc.gpsimd.dma_start(out=tb[t][:], in_=ext[:, :])
            for t in _C0_TABLES:
                nc.gpsimd.dma_start(out=tb[t][:], in_=c0_ext[t][:, :])

            # ---------------- per-sequence staging ----------------
            seq_tiles = {}
            ewin = {}
            worder = [(nn, ww) for nn in range(n_seq) for ww in range(NWIN)]

            def issue_loads(gi):
                if gi >= len(worder) or gi in ewin:
                    return
                nn, ww = worder[gi]
                tl = {}
                for nm, ext in (("kw", k_ext), ("qw", q_ext), ("q2w", q2_ext), ("vw", v_ext)):
                    tl[nm] = win.tile([C, CPW * D], F32, tag=nm, name=nm)
                    nc.sync.dma_start(
                        out=tl[nm][:],
                        in_=ext[nn, ww * CPW : (ww + 1) * CPW, :, :].rearrange("c p d -> p c d"),
                    )
                ewin[gi] = tl

            def get_seq(n):
                if n in seq_tiles:
                    return seq_tiles[n]
                t = {}
                for key, shp2 in [
                    ("qt", [D, l_eff]), ("qtc", [D, l_eff]), ("qts", [D, l_eff]),
                    ("kf", [D, l_eff]), ("kcf", [D, l_eff]), ("ksf", [D, l_eff]),
                    ("klm", [C, nch * D]), ("kcl", [C, nch * D]), ("ksl", [C, nch * D]),
                    ("qel", [D, l_eff]), ("nfq", [D, l_eff]),
                ]:
                    t[key] = sq_.tile(shp2, F16, tag=f"{key}_st", name=f"{key}_st")
                t["vw16"] = sq_.tile([C, nch * VST], F16, tag="vw16_st", name="vw16_st")
                seq_tiles[n] = t
                v3 = t["vw16"][:].rearrange("p (c v) -> p c v", v=VST)
                nc.gpsimd.memset(v3[:, :, D : D + 1], 1.0)
                return t

            def emit_E(n, w):
                t = get_seq(n)
                gi = n * NWIN + w
                issue_loads(gi)
                issue_loads(gi + 2)
                tl = ewin.pop(gi)
                kw, qw, q2w, vw = tl["kw"], tl["qw"], tl["q2w"], tl["vw"]
                wsl = bass.ds(w * W, W)
                wdl = bass.ds(w * CPW * D, CPW * D)

                # v cast into per-seq fp16 staging (ones cols pre-set);
                # per-chunk contiguous casts keep the DVE in 2x mode
                for cc in range(CPW):
                    nc.vector.tensor_scalar(
                        t["vw16"][:, bass.ds((w * CPW + cc) * VST, D)],
                        vw[:, bass.ds(cc * D, D)],
                        0.0, None, OP.add,
                    )

                # K path: klm = min(exp(k),1) + max(k,0)
                ek = win.tile([C, CPW * D], F16, tag="ek")
                nc.scalar.activation(ek[:], kw[:], AF.Exp)
                em = win.tile([C, CPW * D], F16, tag="em")
                nc.vector.tensor_scalar(em[:], ek[:], 1.0, None, OP.min)
                rk = win.tile([C, CPW * D], F16, tag="rk")
                nc.scalar.activation(rk[:], kw[:], AF.Relu)
                nc.vector.tensor_tensor(t["klm"][:, wdl], em[:], rk[:], OP.add)

                # kf via batched PE transposes into one PSUM bank, single evac
                ptk = ptf.tile([D, W], F16, tag="ptf", name="ptk")
                for cc in range(CPW):
                    nc.tensor.transpose(ptk[:, bass.ds(cc * C, C)], t["klm"][:, bass.ds((w * CPW + cc) * D, D)], id16[:])
                nc.vector.tensor_copy(t["kf"][:, wsl], ptk[:])
                nc.vector.tensor_tensor(t["kcf"][:, wsl], t["kf"][:, wsl], tb["c2t_fm"][:, wsl], OP.mult)

                # Q elu (half): qel = min(exp(q)/2, 1/2) + max(q,0)*0.5
                eq = win.tile([C, CPW * D], F16, tag="eq")
                nc.scalar.activation(eq[:], qw[:], AF.Exp, bias=nln2_col[:, 0:1])
                eh2 = win.tile([C, CPW * D], F16, tag="eh2")
                nc.vector.tensor_scalar(eh2[:], eq[:], 0.5, None, OP.min)
                rqh = win.tile([C, CPW * D], F16, tag="rqh")
                nc.scalar.activation(rqh[:], qw[:], AF.Relu, scale=0.5)
                qel_w = win.tile([C, CPW * D], F16, tag="qel_w")
                nc.vector.tensor_tensor(qel_w[:], eh2[:], rqh[:], OP.add)

                # q2 cast to fp16
                q2c = win.tile([C, CPW * D], F16, tag="q2c")
                nc.vector.tensor_scalar(q2c[:], q2w[:], 0.0, None, OP.add)

                # batched fp16 PE transposes; one fused evac per tensor
                ptq = ptf.tile([D, W], F16, tag="ptf", name="ptq")
                for cc in range(CPW):
                    nc.tensor.transpose(ptq[:, bass.ds(cc * C, C)], qel_w[:, bass.ds(cc * D, D)], id16[:])
                nc.scalar.activation(t["qel"][:, wsl], ptq[:], AF.Copy)
                ptq2 = ptf.tile([D, W], F16, tag="ptf", name="ptq2")
                q2f = win.tile([D, W], F16, tag="q2f")
                for cc in range(CPW):
                    nc.tensor.transpose(ptq2[:, bass.ds(cc * C, C)], q2c[:, bass.ds(cc * D, D)], id16[:])
                nc.vector.tensor_copy(q2f[:], ptq2[:])

                # q2 projection (fp16) + magic range reduction -> nfq in [-.5,.5]
                yp = pq2.tile([D, W], F32, tag="q2p")
                nc.tensor.matmul(yp[:], omega_t[:], q2f[:], start=True, stop=True)
                kq = win.tile([D, W], F32, tag="kq")
                nc.scalar.activation(kq[:], yp[:], AF.Identity, bias=magic_col[:, 0:1])
                nc.vector.scalar_tensor_tensor(t["nfq"][:, wsl], kq[:], MAGIC, yp[:], OP.subtract, OP.subtract)

            def emit_E_tail(n):
                t = get_seq(n)
                nc.gpsimd.tensor_tensor(t["kcl"][:], t["klm"][:], tb["c2t_lm"][:], OP.mult)
                nc.gpsimd.tensor_tensor(t["ksl"][:], t["klm"][:], tb["s2t_lm"][:], OP.mult)
                nc.gpsimd.tensor_tensor(t["ksf"][:], t["kf"][:], tb["s2t_fm"][:], OP.mult)

            def emit_T(n, w):
                t = get_seq(n)
                wsl = bass.ds(w * W, W)
                sqw = win.tile([D, W], F16, tag="sqw")
                nc.scalar.activation(sqw[:], t["nfq"][:, wsl], AF.Sin, scale=-TWO_PI)
                s2w = win.tile([D, W], F16, tag="s2w")
                nc.scalar.activation(s2w[:], sqw[:], AF.Square)
                nc.vector.tensor_tensor(t["qt"][:, wsl], s2w[:], t["qel"][:, wsl], OP.mult)

            def emit_qtcs(n, half):
                t = get_seq(n)
                hl = bass.ds(half * (L // 2), L // 2)
                nc.gpsimd.tensor_tensor(t["qtc"][:, hl], t["qt"][:, hl], tb["c2n_fm"][:, hl], OP.mult)
                nc.gpsimd.tensor_tensor(t["qts"][:, hl], t["qt"][:, hl], tb["s2n_fm"][:, hl], OP.mult)

            scan_state = {}

            def emit_scan(n, chunks):
                t = seq_tiles[n]
                st = scan_state.setdefault(n, {"st_ps": None, "sst": None, "ob4": None})
                for c in chunks:
                    first, last = c == 0, c == nch - 1
                    cc = c % CPW
                    sl = bass.ts(c, C)
                    dsl = bass.ts(c, D)
                    vp = t["vw16"][:, bass.ds(c * VST, DV1)]

                    p_ps = pP.tile([C, C], F32, tag="P")
                    if first:
                        st["st_ps"] = pS.tile([D, 3 * SW], F32, tag="st", name="st_ps")
                        qa = wk.tile([D, C], F32, tag="qa")
                        nc.gpsimd.tensor_tensor(qa[:], t["qt"][:, 0:C], tb["qs2_0"][:], OP.mult)
                        qb = wk.tile([D, C], F32, tag="qb")
                        nc.gpsimd.tensor_tensor(qb[:], t["qt"][:, 0:C], tb["qc2_0"][:], OP.mult)
                        qc = wk.tile([D, C], F32, tag="qc")
                        nc.gpsimd.tensor_tensor(qc[:], t["qt"][:, 0:C], tb["qsc_0"][:], OP.mult)
                        ka = wk.tile([D, C], F32, tag="ka")
                        nc.gpsimd.tensor_tensor(ka[:], t["kf"][:, 0:C], tb["kc2_0"][:], OP.mult)
                        kb = wk.tile([D, C], F32, tag="kb")
                        nc.gpsimd.tensor_tensor(kb[:], t["kf"][:, 0:C], tb["ks2_0"][:], OP.mult)
                        kc = wk.tile([D, C], F32, tag="kc")
                        nc.gpsimd.tensor_tensor(kc[:], t["kf"][:, 0:C], tb["ksc_0"][:], OP.mult)
                        nc.tensor.matmul(p_ps[:], ka[:], qa[:], start=True, stop=False)
                        nc.tensor.matmul(p_ps[:], kb[:], qb[:], start=False, stop=False)
                        nc.tensor.matmul(p_ps[:], kc[:], qc[:], start=False, stop=True)
                    else:
                        nc.tensor.matmul(p_ps[:], t["kf"][:, sl], t["qt"][:, sl], start=True, stop=False)
                        nc.tensor.matmul(p_ps[:], t["kcf"][:, sl], t["qtc"][:, sl], start=False, stop=False)
                        nc.tensor.matmul(p_ps[:], t["ksf"][:, sl], t["qts"][:, sl], start=False, stop=True)

                    p_sb = wk.tile([C, C], F16, tag="p_sb")
                    nc.vector.tensor_tensor(p_sb[:], p_ps[:], mask_sb[:], OP.mult)

                    o_ps = pO.tile([C, DV1], F32, tag="O")
                    nc.tensor.matmul(o_ps[:], p_sb[:], vp, start=True, stop=first)
                    if not first:
                        sst = st["sst"]
                        nc.tensor.matmul(o_ps[:], t["qt"][:, sl], sst[:, 0:DV1], start=False, stop=False)
                        nc.tensor.matmul(o_ps[:], t["qtc"][:, sl], sst[:, SW : SW + DV1], start=False, stop=False)
                        nc.tensor.matmul(o_ps[:], t["qts"][:, sl], sst[:, 2 * SW : 2 * SW + DV1], start=False, stop=True)

                    if not last:
                        sp = st["st_ps"]
                        nc.tensor.matmul(sp[:, 0:DV1], t["klm"][:, dsl], vp, start=first, stop=True, skip_group_check=not first)
                        nc.tensor.matmul(sp[:, SW : SW + DV1], t["kcl"][:, dsl], vp, start=False, stop=True, skip_group_check=True)
                        nc.tensor.matmul(sp[:, 2 * SW : 2 * SW + DV1], t["ksl"][:, dsl], vp, start=False, stop=True, skip_group_check=True)
                        sst = wk.tile([D, 3 * SW], F16, tag="sst")
                        nc.scalar.activation(sst[:], sp[:], AF.Copy)
                        st["sst"] = sst

                    if cc == 0:
                        st["ob4"] = op_.tile([C, CPW * DV1], F16, tag="ob4", name="ob4")
                    nc.scalar.activation(st["ob4"][:, bass.ds(cc * DV1, DV1)], o_ps[:, 0:DV1], AF.Copy, scale=0.0625)
                    if cc == CPW - 1:
                        nc.gpsimd.dma_start(out=out_ext[n, c // CPW, :, :], in_=st["ob4"][:])

            # ---------------- emission schedule ----------------
            def t_phase(n):
                emit_T(n, 0)
                emit_T(n, 1)
                emit_qtcs(n, 0)
                emit_scan(n, [0, 1, 2])
                emit_T(n, 2)
                emit_scan(n, [3, 4, 5])
                emit_T(n, 3)
                emit_qtcs(n, 1)
                emit_scan(n, [6, 7, 8])
                emit_scan(n, [9, 10, 11])

            issue_loads(0)
            issue_loads(1)
            for w in range(NWIN):
                emit_E(0, w)
            emit_E_tail(0)
            t_phase(0)
            for n in range(1, n_seq):
                for w in range(NWIN):
                    emit_scan(n - 1, [12 + w])
                    emit_E(n, w)
                emit_E_tail(n)
                del seq_tiles[n - 1]
                t_phase(n)
            emit_scan(n_seq - 1, list(range(12, nch)))

    nc.finalize()
    return nc


def _host_tables(om_h):
    """Trig tables for one head from omega [D, D] (float64 math)."""
    om64 = om_h.astype(np.float64)
    w = om64.sum(axis=0)  # w[j] = sum_i omega[i, j]
    t = np.outer(w, np.arange(L, dtype=np.float64) / L)  # [D, L]
    s, c = np.sin(t), np.cos(t)
    s0, c0 = s[:, :C], c[:, :C]
    c2, s2 = np.cos(2.0 * t), np.sin(2.0 * t)
    lm = lambda x: np.ascontiguousarray(
        x.reshape(D, NCH, C).transpose(2, 1, 0).reshape(C, NCH * D)
    )
    return {
        "omega16": (om64 / TWO_PI).astype(np.float16),
        "qs2_0": (s0**2).astype(np.float32),
        "qc2_0": (c0**2).astype(np.float32),
        "qsc_0": (-2.0 * s0 * c0).astype(np.float32),
        "kc2_0": (2.0 * c0**2).astype(np.float32),
        "ks2_0": (2.0 * s0**2).astype(np.float32),
        "ksc_0": (2.0 * s0 * c0).astype(np.float32),
        "c2t_fm": c2.astype(np.float16),
        "s2t_fm": s2.astype(np.float16),
        "c2n_fm": (-c2).astype(np.float16),
        "s2n_fm": (-s2).astype(np.float16),
        "c2t_lm": lm(c2).astype(np.float16),
        "s2t_lm": lm(s2).astype(np.float16),
    }


def _host_inputs(inputs, n_seq=N, nch=NCH):
    l_eff = nch * C
    q = np.ascontiguousarray(inputs["queries"], dtype=np.float32)
    q2 = np.ascontiguousarray(inputs["q2"], dtype=np.float32)
    k = np.ascontiguousarray(inputs["keys"], dtype=np.float32)
    v = np.ascontiguousarray(inputs["values"], dtype=np.float32)
    om = np.ascontiguousarray(inputs["omega"], dtype=np.float32)

    mask = np.triu(np.ones((C, C), dtype=np.float16))

    def shp(x, h):
        return np.ascontiguousarray(x[:n_seq, :l_eff, h, :]).reshape(n_seq, nch, C, D)

    in_maps = []
    for h in range(om.shape[0] if om.ndim == 3 else H):
        m = {
            "queries": shp(q, h),
            "q2": shp(q2, h),
            "keys": shp(k, h),
            "values": shp(v, h),
            "mask": mask,
        }
        m.update(_host_tables(om[h]))
        in_maps.append(m)
    return in_maps


def _run(inputs, trace=False):
    if "nc" not in _CACHE:
        _CACHE["nc"] = build_nc()
    nc = _CACHE["nc"]
    in_maps = _host_inputs(inputs)
    res = run_bass_kernel_spmd(nc, in_maps, core_ids=list(range(H)), trace=trace)
    outs = []
    for hh in range(H):
        o = res.results[hh]["out"].reshape(N, NWIN, C, CPW, DV1).astype(np.float32)
        num, z = o[..., :D], o[..., D]
        o = num / (z + EPS * 0.0625)[..., None]
        outs.append(o.transpose(0, 1, 3, 2, 4).reshape(N, L, D))
    full = np.stack(outs, axis=2)
    return full.astype(np.float32), res


def kernel(**inputs):
    out, _ = _run(inputs, trace=False)
    return out


# revision 20
# speedup vs baseline: 1.3275x; 1.1241x over previous
"""v6: host-precomputed trig tables + vector-lean feature pipeline.

Structure (per core = one head, 4 sequences):
  - All trig tables (double-angle FM/LM fp16, chunk-0 exact fp32, omega/2pi
    fp16) are computed host-side from omega and DMA'd in; no on-device
    table generation.
  - Feature phase split per sequence into an Exp pass (E) and Trig pass (T)
    so the scalar engine loads each activation table set once per sequence.
  - elu1 fused as min(exp(x),1)+max(x,0) via TS(min)+TT(add); exp reads the
    raw fp32 window directly (inf clamps through min).
  - fp32->fp16 casts via tensor_scalar (tensor_copy falls to 1x mode).
  - GpSimd: sequence-wide TTs for ksl/ksf/qts/qtc + chunk-0 branch TTs.
  - Scan: 3 branch P matmuls -> masked p_sb -> intra/inter O matmuls; 3
    states share one PSUM bank with a single fused fp16 evacuation.
  - Scan of seq n-1 interleaved 2 chunks per feature sub-slot of seq n.
"""

import math

import numpy as np

import concourse.bass as bass
import concourse.tile as tile
from concourse import bacc, mybir
from concourse.bass_utils import run_bass_kernel_spmd
from concourse.masks import make_identity

F32 = mybir.dt.float32
F16 = mybir.dt.float16
AF = mybir.ActivationFunctionType
OP = mybir.AluOpType

N, L, H, D = 4, 2048, 8, 128
C = 128
NCH = L // C
DV1 = D + 1
VST = 130  # per-chunk slot width in v staging (4B-aligned stride)
SW = 130  # state slice stride in PSUM cols (8B aligned)
TWO_PI = 2.0 * math.pi
MAGIC = float(np.float32(1.5 * 2**23))
LN2 = float(np.log(2.0))
EPS = 1e-6
W = 512
CPW = W // C
NWIN = L // W

_CACHE = {}

_FM_TABLES = ["c2t_fm", "s2t_fm", "c2n_fm", "s2n_fm"]
_LM_TABLES = ["c2t_lm", "s2t_lm"]
_C0_TABLES = ["qs2_0", "qc2_0", "qsc_0", "kc2_0", "ks2_0", "ksc_0"]


def build_nc(n_seq=N, nch=NCH):
    l_eff = nch * C
    nc = bacc.Bacc(None, target_bir_lowering=False, debug=False)

    q_ext = nc.declare_dram_parameter("queries", [n_seq, nch, C, D], F32, isOutput=False)
    q2_ext = nc.declare_dram_parameter("q2", [n_seq, nch, C, D], F32, isOutput=False)
    k_ext = nc.declare_dram_parameter("keys", [n_seq, nch, C, D], F32, isOutput=False)
    v_ext = nc.declare_dram_parameter("values", [n_seq, nch, C, D], F32, isOutput=False)
    om_ext = nc.declare_dram_parameter("omega16", [D, D], F16, isOutput=False)
    mask_ext = nc.declare_dram_parameter("mask", [C, C], F16, isOutput=False)
    fm_ext = {t: nc.declare_dram_parameter(t, [D, l_eff], F16, isOutput=False) for t in _FM_TABLES}
    lm_ext = {t: nc.declare_dram_parameter(t, [C, nch * D], F16, isOutput=False) for t in _LM_TABLES}
    c0_ext = {t: nc.declare_dram_parameter(t, [D, C], F32, isOutput=False) for t in _C0_TABLES}
    out_ext = nc.declare_dram_parameter("out", [n_seq, NWIN, C, CPW * DV1], F16, isOutput=True)

    with tile.TileContext(nc) as tc:
        with (
            tc.tile_pool(name="persist", bufs=1) as pp,
            tc.tile_pool(name="seqst", bufs=2) as sq_,
            tc.tile_pool(name="win", bufs=3) as win,
            tc.tile_pool(name="drs", bufs=2, space="DRAM") as drs,
            tc.tile_pool(name="work", bufs=3) as wk,
            tc.tile_pool(name="outp", bufs=2) as op_,
            tc.tile_pool(name="ptf", bufs=2, space="PSUM") as ptf,
            tc.tile_pool(name="pq2", bufs=1, space="PSUM") as pq2,
            tc.tile_pool(name="pP", bufs=2, space="PSUM") as pP,
            tc.tile_pool(name="pO", bufs=2, space="PSUM") as pO,
            tc.tile_pool(name="pS", bufs=1, space="PSUM") as pS,
        ):
            # ---------------- one-time setup (DMA only) ----------------
            id16 = pp.tile([D, D], F16, tag="id16")
            make_identity(nc, id16[:])
            magic_col = pp.tile([D, 1], F32, tag="magic")
            nc.gpsimd.memset(magic_col[:], MAGIC)
            nln2_col = pp.tile([D, 1], F32, tag="nln2")
            nc.gpsimd.memset(nln2_col[:], -LN2)

            omega_t = pp.tile([D, D], F16, tag="omega_t")
            nc.gpsimd.dma_start(out=omega_t[:], in_=om_ext[:, :])
            mask_sb = pp.tile([C, C], F16, tag="mask")
            nc.gpsimd.dma_start(out=mask_sb[:], in_=mask_ext[:, :])
            tb = {}
            for t in _FM_TABLES:
                tb[t] = pp.tile([D, l_eff], F16, tag=t, name=t)
            for t in _LM_TABLES:
                tb[t] = pp.tile([C, nch * D], F16, tag=t, name=t)
            for t in _C0_TABLES:
                tb[t] = pp.tile([D, C], F32, tag=t, name=t)
            nc.gpsimd.dma_start(out=tb["c2t_fm"][:], in_=fm_ext["c2t_fm"][:, :])

            def emit_tables(stage):
                if stage == 1:
                    for t in ["c2t_lm", "s2t_lm", "s2t_fm"]:
                        ext = fm_ext[t] if t in fm_ext else lm_ext[t]
                        nc.gpsimd.dma_start(out=tb[t][:], in_=ext[:, :])
                else:
                    for t in ["c2n_fm", "s2n_fm"]:
                        nc.gpsimd.dma_start(out=tb[t][:], in_=fm_ext[t][:, :])
                    for t in _C0_TABLES:
                        nc.gpsimd.dma_start(out=tb[t][:], in_=c0_ext[t][:, :])

            # ---------------- per-sequence staging ----------------
            seq_tiles = {}
            ewin = {}
            worder = [(nn, ww) for nn in range(n_seq) for ww in range(NWIN)]

            def issue_loads(gi):
                if gi >= len(worder) or gi in ewin:
                    return
                nn, ww = worder[gi]
                tl = {}
                for nm, ext in (("kw", k_ext), ("qw", q_ext), ("q2w", q2_ext), ("vw", v_ext)):
                    tl[nm] = win.tile([C, CPW * D], F32, tag=nm, name=nm)
                    nc.sync.dma_start(
                        out=tl[nm][:],
                        in_=ext[nn, ww * CPW : (ww + 1) * CPW, :, :].rearrange("c p d -> p c d"),
                    )
                ewin[gi] = tl

            def get_seq(n):
                if n in seq_tiles:
                    return seq_tiles[n]
                t = {}
                for key, shp2 in [
                    ("qt", [D, l_eff]), ("qtc", [D, l_eff]), ("qts", [D, l_eff]),
                    ("kf", [D, l_eff]), ("kcf", [D, l_eff]), ("ksf", [D, l_eff]),
                    ("klm", [C, nch * D]), ("kcl", [C, nch * D]), ("ksl", [C, nch * D]),
                    ("qel", [D, l_eff]), ("nfq", [D, l_eff]),
                ]:
                    t[key] = sq_.tile(shp2, F16, tag=f"{key}_st", name=f"{key}_st")
                t["vw16"] = sq_.tile([C, nch * VST], F16, tag="vw16_st", name="vw16_st")
                seq_tiles[n] = t
                v3 = t["vw16"][:].rearrange("p (c v) -> p c v", v=VST)
                nc.gpsimd.memset(v3[:, :, D : D + 1], 1.0)
                return t

            def emit_E(n, w):
                t = get_seq(n)
                gi = n * NWIN + w
                issue_loads(gi)
                issue_loads(gi + 2)
                tl = ewin.pop(gi)
                kw, qw, q2w, vw = tl["kw"], tl["qw"], tl["q2w"], tl["vw"]
                wsl = bass.ds(w * W, W)
                wdl = bass.ds(w * CPW * D, CPW * D)

                # v cast into per-seq fp16 staging (ones cols pre-set);
                # per-chunk contiguous casts keep the DVE in 2x mode
                for cc in range(CPW):
                    nc.vector.tensor_scalar(
                        t["vw16"][:, bass.ds((w * CPW + cc) * VST, D)],
                        vw[:, bass.ds(cc * D, D)],
                        0.0, None, OP.add,
                    )

                # K path: klm = min(exp(k),1) + max(k,0)
                ek = win.tile([C, CPW * D], F16, tag="ek")
                nc.scalar.activation(ek[:], kw[:], AF.Exp)
                em = win.tile([C, CPW * D], F16, tag="em")
                nc.vector.tensor_scalar(em[:], ek[:], 1.0, None, OP.min)
                rk = win.tile([C, CPW * D], F16, tag="rk")
                nc.scalar.activation(rk[:], kw[:], AF.Relu)
                nc.vector.tensor_tensor(t["klm"][:, wdl], em[:], rk[:], OP.add)

                # kf via batched PE transposes into one PSUM bank, single evac
                ptk = ptf.tile([D, W], F16, tag="ptf", name="ptk")
                for cc in range(CPW):
                    nc.tensor.transpose(ptk[:, bass.ds(cc * C, C)], t["klm"][:, bass.ds((w * CPW + cc) * D, D)], id16[:])
                nc.vector.tensor_copy(t["kf"][:, wsl], ptk[:])
                nc.vector.tensor_tensor(t["kcf"][:, wsl], t["kf"][:, wsl], tb["c2t_fm"][:, wsl], OP.mult)

                # Q elu (half): qel = min(exp(q)/2, 1/2) + max(q,0)*0.5
                eq = win.tile([C, CPW * D], F16, tag="eq")
                nc.scalar.activation(eq[:], qw[:], AF.Exp, bias=nln2_col[:, 0:1])
                eh2 = win.tile([C, CPW * D], F16, tag="eh2")
                nc.vector.tensor_scalar(eh2[:], eq[:], 0.5, None, OP.min)
                rqh = win.tile([C, CPW * D], F16, tag="rqh")
                nc.scalar.activation(rqh[:], qw[:], AF.Relu, scale=0.5)
                qel_w = win.tile([C, CPW * D], F16, tag="qel_w")
                nc.vector.tensor_tensor(qel_w[:], eh2[:], rqh[:], OP.add)

                # q2 cast to fp16
                q2c = win.tile([C, CPW * D], F16, tag="q2c")
                nc.vector.tensor_scalar(q2c[:], q2w[:], 0.0, None, OP.add)

                # batched fp16 PE transposes; one fused evac per tensor
                ptq = ptf.tile([D, W], F16, tag="ptf", name="ptq")
                for cc in range(CPW):
                    nc.tensor.transpose(ptq[:, bass.ds(cc * C, C)], qel_w[:, bass.ds(cc * D, D)], id16[:])
                nc.scalar.activation(t["qel"][:, wsl], ptq[:], AF.Copy)
                ptq2 = ptf.tile([D, W], F16, tag="ptf", name="ptq2")
                q2f = win.tile([D, W], F16, tag="q2f")
                for cc in range(CPW):
                    nc.tensor.transpose(ptq2[:, bass.ds(cc * C, C)], q2c[:, bass.ds(cc * D, D)], id16[:])
                nc.vector.tensor_copy(q2f[:], ptq2[:])

                # q2 projection (fp16) + magic range reduction -> nfq in [-.5,.5]
                yp = pq2.tile([D, W], F32, tag="q2p")
                nc.tensor.matmul(yp[:], omega_t[:], q2f[:], start=True, stop=True)
                kq = win.tile([D, W], F32, tag="kq")
                nc.scalar.activation(kq[:], yp[:], AF.Identity, bias=magic_col[:, 0:1])
                nc.vector.scalar_tensor_tensor(t["nfq"][:, wsl], kq[:], MAGIC, yp[:], OP.subtract, OP.subtract)

            def emit_E_tail(n):
                t = get_seq(n)
                nc.gpsimd.tensor_tensor(t["kcl"][:], t["klm"][:], tb["c2t_lm"][:], OP.mult)
                nc.gpsimd.tensor_tensor(t["ksl"][:], t["klm"][:], tb["s2t_lm"][:], OP.mult)
                nc.gpsimd.tensor_tensor(t["ksf"][:], t["kf"][:], tb["s2t_fm"][:], OP.mult)

            def emit_T(n, w):
                t = get_seq(n)
                wsl = bass.ds(w * W, W)
                sqw = win.tile([D, W], F16, tag="sqw")
                nc.scalar.activation(sqw[:], t["nfq"][:, wsl], AF.Sin, scale=-TWO_PI)
                s2w = win.tile([D, W], F16, tag="s2w")
                nc.scalar.activation(s2w[:], sqw[:], AF.Square)
                nc.vector.tensor_tensor(t["qt"][:, wsl], s2w[:], t["qel"][:, wsl], OP.mult)

            def emit_qtcs(n, half):
                t = get_seq(n)
                hl = bass.ds(half * (L // 2), L // 2)
                nc.gpsimd.tensor_tensor(t["qtc"][:, hl], t["qt"][:, hl], tb["c2n_fm"][:, hl], OP.mult)
                nc.gpsimd.tensor_tensor(t["qts"][:, hl], t["qt"][:, hl], tb["s2n_fm"][:, hl], OP.mult)

            scan_state = {}

            def emit_scan(n, chunks, sv_vec=False):
                t = seq_tiles[n]
                st = scan_state.setdefault(n, {"st_ps": None, "sst": None, "ob4": None})
                for c in chunks:
                    first, last = c == 0, c == nch - 1
                    cc = c % CPW
                    sl = bass.ts(c, C)
                    dsl = bass.ts(c, D)
                    vp = t["vw16"][:, bass.ds(c * VST, DV1)]

                    p_ps = pP.tile([C, C], F32, tag="P")
                    if first:
                        st["st_ps"] = pS.tile([D, 3 * SW], F32, tag="st", name="st_ps")
                        qa = wk.tile([D, C], F32, tag="qa")
                        nc.gpsimd.tensor_tensor(qa[:], t["qt"][:, 0:C], tb["qs2_0"][:], OP.mult)
                        qb = wk.tile([D, C], F32, tag="qb")
                        nc.gpsimd.tensor_tensor(qb[:], t["qt"][:, 0:C], tb["qc2_0"][:], OP.mult)
                        qc = wk.tile([D, C], F32, tag="qc")
                        nc.gpsimd.tensor_tensor(qc[:], t["qt"][:, 0:C], tb["qsc_0"][:], OP.mult)
                        ka = wk.tile([D, C], F32, tag="ka")
                        nc.gpsimd.tensor_tensor(ka[:], t["kf"][:, 0:C], tb["kc2_0"][:], OP.mult)
                        kb = wk.tile([D, C], F32, tag="kb")
                        nc.gpsimd.tensor_tensor(kb[:], t["kf"][:, 0:C], tb["ks2_0"][:], OP.mult)
                        kc = wk.tile([D, C], F32, tag="kc")
                        nc.gpsimd.tensor_tensor(kc[:], t["kf"][:, 0:C], tb["ksc_0"][:], OP.mult)
                        nc.tensor.matmul(p_ps[:], ka[:], qa[:], start=True, stop=False)
                        nc.tensor.matmul(p_ps[:], kb[:], qb[:], start=False, stop=False)
                        nc.tensor.matmul(p_ps[:], kc[:], qc[:], start=False, stop=True)
                    else:
                        nc.tensor.matmul(p_ps[:], t["kf"][:, sl], t["qt"][:, sl], start=True, stop=False)
                        nc.tensor.matmul(p_ps[:], t["kcf"][:, sl], t["qtc"][:, sl], start=False, stop=False)
                        nc.tensor.matmul(p_ps[:], t["ksf"][:, sl], t["qts"][:, sl], start=False, stop=True)

                    p_sb = wk.tile([C, C], F16, tag="p_sb")
                    nc.vector.tensor_tensor(p_sb[:], p_ps[:], mask_sb[:], OP.mult)

                    o_ps = pO.tile([C, DV1], F32, tag="O")
                    nc.tensor.matmul(o_ps[:], p_sb[:], vp, start=True, stop=first)
                    if not first:
                        sst = st["sst"]
                        nc.tensor.matmul(o_ps[:], t["qt"][:, sl], sst[:, 0:DV1], start=False, stop=False)
                        nc.tensor.matmul(o_ps[:], t["qtc"][:, sl], sst[:, SW : SW + DV1], start=False, stop=False)
                        nc.tensor.matmul(o_ps[:], t["qts"][:, sl], sst[:, 2 * SW : 2 * SW + DV1], start=False, stop=True)

                    if not last:
                        sp = st["st_ps"]
                        nc.tensor.matmul(sp[:, 0:DV1], t["klm"][:, dsl], vp, start=first, stop=True, skip_group_check=not first)
                        nc.tensor.matmul(sp[:, SW : SW + DV1], t["kcl"][:, dsl], vp, start=False, stop=True, skip_group_check=True)
                        nc.tensor.matmul(sp[:, 2 * SW : 2 * SW + DV1], t["ksl"][:, dsl], vp, start=False, stop=True, skip_group_check=True)
                        sst = wk.tile([D, 3 * SW], F16, tag="sst")
                        if sv_vec:
                            nc.vector.tensor_copy(sst[:], sp[:])
                        else:
                            nc.scalar.activation(sst[:], sp[:], AF.Copy)
                        st["sst"] = sst

                    if cc == 0:
                        st["ob4"] = op_.tile([C, CPW * DV1], F16, tag="ob4", name="ob4")
                    nc.scalar.activation(st["ob4"][:, bass.ds(cc * DV1, DV1)], o_ps[:, 0:DV1], AF.Copy, scale=0.0625)
                    if cc == CPW - 1:
                        nc.gpsimd.dma_start(out=out_ext[n, c // CPW, :, :], in_=st["ob4"][:])

            # ---------------- emission schedule ----------------
            def t_phase(n):
                emit_T(n, 0)
                emit_T(n, 1)
                emit_qtcs(n, 0)
                emit_scan(n, [0, 1, 2], sv_vec=True)
                emit_T(n, 2)
                emit_scan(n, [3, 4, 5], sv_vec=True)
                emit_T(n, 3)
                emit_qtcs(n, 1)
                emit_scan(n, [6, 7, 8], sv_vec=True)
                emit_scan(n, [9, 10, 11], sv_vec=True)

            issue_loads(0)
            issue_loads(1)
            for w in range(NWIN):
                emit_E(0, w)
                if w == 0:
                    emit_tables(1)
                elif w == 1:
                    emit_tables(2)
            emit_E_tail(0)
            t_phase(0)
            for n in range(1, n_seq):
                for w in range(NWIN):
                    emit_scan(n - 1, [12 + w])
                    emit_E(n, w)
                emit_E_tail(n)
                del seq_tiles[n - 1]
                t_phase(n)
            emit_scan(n_seq - 1, list(range(12, nch)))

    nc.finalize()
    return nc


def _host_tables(om_h):
    """Trig tables for one head from omega [D, D] (float64 math)."""
    om64 = om_h.astype(np.float64)
    w = om64.sum(axis=0)  # w[j] = sum_i omega[i, j]
    t = np.outer(w, np.arange(L, dtype=np.float64) / L)  # [D, L]
    s, c = np.sin(t), np.cos(t)
    s0, c0 = s[:, :C], c[:, :C]
    c2, s2 = np.cos(2.0 * t), np.sin(2.0 * t)
    lm = lambda x: np.ascontiguousarray(
        x.reshape(D, NCH, C).transpose(2, 1, 0).reshape(C, NCH * D)
    )
    return {
        "omega16": (om64 / TWO_PI).astype(np.float16),
        "qs2_0": (s0**2).astype(np.float32),
        "qc2_0": (c0**2).astype(np.float32),
        "qsc_0": (-2.0 * s0 * c0).astype(np.float32),
        "kc2_0": (2.0 * c0**2).astype(np.float32),
        "ks2_0": (2.0 * s0**2).astype(np.float32),
        "ksc_0": (2.0 * s0 * c0).astype(np.float32),
        "c2t_fm": c2.astype(np.float16),
        "s2t_fm": s2.astype(np.float16),
        "c2n_fm": (-c2).astype(np.float16),
        "s2n_fm": (-s2).astype(np.float16),
        "c2t_lm": lm(c2).astype(np.float16),
        "s2t_lm": lm(s2).astype(np.float16),
    }


def _host_inputs(inputs, n_seq=N, nch=NCH):
    l_eff = nch * C
    q = np.ascontiguousarray(inputs["queries"], dtype=np.float32)
    q2 = np.ascontiguousarray(inputs["q2"], dtype=np.float32)
    k = np.ascontiguousarray(inputs["keys"], dtype=np.float32)
    v = np.ascontiguousarray(inputs["values"], dtype=np.float32)
    om = np.ascontiguousarray(inputs["omega"], dtype=np.float32)

    mask = np.triu(np.ones((C, C), dtype=np.float16))

    def shp(x, h):
        return np.ascontiguousarray(x[:n_seq, :l_eff, h, :]).reshape(n_seq, nch, C, D)

    in_maps = []
    for h in range(om.shape[0] if om.ndim == 3 else H):
        m = {
            "queries": shp(q, h),
            "q2": shp(q2, h),
            "keys": shp(k, h),
            "values": shp(v, h),
            "mask": mask,
        }
        m.update(_host_tables(om[h]))
        in_maps.append(m)
    return in_maps


def _run(inputs, trace=False):
    if "nc" not in _CACHE:
        _CACHE["nc"] = build_nc()
    nc = _CACHE["nc"]
    in_maps = _host_inputs(inputs)
    res = run_bass_kernel_spmd(nc, in_maps, core_ids=list(range(H)), trace=trace)
    outs = []
    for hh in range(H):
        o = res.results[hh]["out"].reshape(N, NWIN, C, CPW, DV1).astype(np.float32)
        num, z = o[..., :D], o[..., D]
        o = num / (z + EPS * 0.0625)[..., None]
        outs.append(o.transpose(0, 1, 3, 2, 4).reshape(N, L, D))
    full = np.stack(outs, axis=2)
    return full.astype(np.float32), res


def kernel(**inputs):
    out, _ = _run(inputs, trace=False)
    return out
